# revision 1
# baseline (speedup 1.0000x reference)
"""Trainium2 Bass kernel for CustomMultiHeadAttention (single-query pooled attention).

Reference computation (B=32, S=1024, D=256, H=8):
    keys   = (x @ Wk + bk).reshape(B,S,H,D)
    values = (x @ Wv + bv).reshape(B,S,H,D)
    scores = einsum('bshd,hd->bsh', keys, query)
    attn   = softmax(scores, axis=1)           # over S
    pooled = einsum('bsh,bshd->bhd', attn, values).reshape(B, H*D)
    out    = pooled @ Wo + bo

Algebraic restructure (exact in real arithmetic):
    q_proj[e,h] = sum_d Wk[e, h*D+d] * query[h,d]        # [256, 8]
    scores[b,s,h] = x[b,s,:] @ q_proj[:,h]  (+ const(h) from bk -> cancels in softmax)
    attnu = exp(scores - 64)                             # const shift; softmax invariant
    ctx[b,h,e]  = sum_s attnu[b,s,h] * x[b,s,e];  Z[b,h] = sum_s attnu[b,s,h]
    pooled[b,h,:] = (ctx[b,h,:]/Z[b,h]) @ Wv_h + bv_h    # sum_s attn = 1
    out = pooled @ Wo + (bv @ Wo + bo)

This removes both [B*S,256]x[256,2048] projections; the kernel is memory-bound.
Z is obtained free as an extra all-ones column appended to x in the ctx matmul.
Scores use exact fp32 matmuls (cheap: N=8); the post-softmax path uses float32r.
Sharding: data-parallel over batch, 4 batches per core on 8 cores.

Layout note: PE matmul operands/outputs need base partition in {0,32,64}, so
local batches 0..2 sit at partition offsets 0/32/64 and batch 3 uses a second
free-dim slab at offset 0 (only relevant for the tiny [8 x *] ctx tiles).
"""

import sys

sys.path.insert(0, "/opt/trn_rl_repo")

import numpy as np

import concourse.bass as bass
import concourse.mybir as mybir
import concourse.tile as tile
from concourse import bacc
from concourse.bass_utils import run_bass_kernel_spmd
from concourse.masks import make_identity

F32 = mybir.dt.float32
F32R = mybir.dt.float32r

B, S, D, H = 32, 1024, 256, 8
NCORES = 8
BL = B // NCORES      # local batches per core = 4
ST = S // 128         # s-tiles per batch = 8
KD = 2                # 256 = 2 k-tiles of 128 over the D (input dim) axis
KHD = (H * D) // 128  # 16 k-tiles over the H*D axis
SHIFT = 64.0          # constant score shift before exp (softmax-invariant)

def build_program():
    nc = bacc.Bacc("TRN2", target_bir_lowering=False, debug=False)

    xn_d = nc.dram_tensor("xn", [BL, S, D + 2], F32R, kind="ExternalInput")
    wk_d = nc.dram_tensor("wk", [D, H * D], F32, kind="ExternalInput")
    wv_d = nc.dram_tensor("wv", [D, H * D], F32R, kind="ExternalInput")
    wo_d = nc.dram_tensor("wo", [H * D, D], F32R, kind="ExternalInput")
    q_d = nc.dram_tensor("q", [H, D], F32, kind="ExternalInput")
    bv_d = nc.dram_tensor("bv", [H * D], F32, kind="ExternalInput")
    bo_d = nc.dram_tensor("bo", [D], F32R, kind="ExternalInput")
    on_d = nc.dram_tensor("on", [1, BL], F32R, kind="ExternalInput")
    out_d = nc.dram_tensor("out", [BL, D], F32, kind="ExternalOutput")

    with tile.TileContext(nc) as tc:
        with (
            tc.tile_pool(name="big", bufs=1) as big,
            tc.tile_pool(name="sm", bufs=1) as sm,
            tc.tile_pool(name="ps", bufs=1, space=bass.MemorySpace.PSUM) as ps,
            tc.tile_pool(name="pst", bufs=2, space=bass.MemorySpace.PSUM) as pst,
        ):
            # ---- SBUF allocations -------------------------------------
            xn_sb = big.tile([128, BL, ST, D + 2], F32R)  # x natural + 2 ones cols
            xt_sb = big.tile([128, KD, BL, S], F32)       # x transposed: p=e%128
            wk_sb = big.tile([128, KD, H * D], F32)
            wv_sb = big.tile([128, KD, H * D], F32R)
            wo_sb = big.tile([128, KHD, D], F32R)
            qrep = big.tile([128, H * D], F32)            # query replicated
            qsmall = sm.tile([1, H * D], F32)
            tmp = big.tile([128, KD, H * D], F32)         # wk * qrep scratch

            qp = sm.tile([128, KD, H], F32)               # q_proj [e, h]
            attn_sb = sm.tile([128, BL, ST, H], F32R)     # exp(scores-SHIFT) [s, h]
            recip = sm.tile([H, BL, 1], F32)              # 1/Z per (h, b)
            ctx_sb = sm.tile([H, BL, D], F32)             # [h, b, e]
            ctxT_sb = sm.tile([128, KD, BL, H], F32R)     # [e%128, eh, b, h]
            pooledT_sb = sm.tile([128, KHD, BL], F32R)    # [(hd)%128, ktile, b]
            bvn_sb = sm.tile([KHD, 128], F32)             # bv natural [k, p]
            bvT_sb = sm.tile([128, KHD], F32R)
            bo_sb = sm.tile([1, D], F32R)
            bias_sb = sm.tile([1, D], F32R)               # bv @ Wo + bo
            ones_sb = sm.tile([1, BL], F32R)
            ident = sm.tile([16, 16], F32)
            ident128 = sm.tile([128, 128], F32)
            negs = sm.tile([128, 1], F32)                 # -SHIFT bias for exp
            out_sb = sm.tile([BL, D], F32)

            # ---- DMA loads -------------------------------------------
            nc.sync.dma_start(
                qsmall[:], q_d[:].rearrange("h d -> () (h d)")
            )
            nc.gpsimd.partition_broadcast(qrep[:], qsmall[:])
            nc.sync.dma_start(
                wk_sb[:], wk_d[:].rearrange("(k p) f -> p k f", p=128)
            )
            for b in range(BL):
                nc.sync.dma_start(
                    xn_sb[:, b, :, :],
                    xn_d[b].rearrange("(t p) e -> p t e", p=128),
                )
            nc.sync.dma_start(
                wv_sb[:], wv_d[:].rearrange("(k p) f -> p k f", p=128)
            )
            for kh in range(2):
                nc.sync.dma_start(
                    wo_sb[:, kh * 8:(kh + 1) * 8, :],
                    wo_d[kh * 1024:(kh + 1) * 1024, :]
                    .rearrange("(k p) n -> p k n", p=128),
                )
            nc.sync.dma_start(bvn_sb[:], bv_d[:].rearrange("(k p) -> k p", p=128))
            nc.sync.dma_start(bo_sb[:], bo_d[:].rearrange("d -> () d"))
            nc.sync.dma_start(ones_sb[:], on_d[:])

            make_identity(nc, ident[:])
            make_identity(nc, ident128[:])
            nc.vector.memset(negs[:], -SHIFT)

            # ---- q_proj[e,h] = sum_d Wk[e, h*D+d] * query[h,d] --------
            nc.vector.tensor_mul(
                tmp[:],
                wk_sb[:],
                qrep[:].rearrange("p f -> p () f").broadcast_to([128, KD, H * D]),
            )
            nc.vector.reduce_sum(
                qp[:],
                tmp[:].rearrange("p k (h d) -> p k h d", d=D),
                axis=mybir.AxisListType.X,
            )

            # ---- transpose x on chip: xt[e, s] per (b, eh) (PE, fp32) -
            # 4 transposes share one PSUM bank -> one batched DVE copy
            for b in range(BL):
                for tp2 in range(ST // 2):       # pairs of s-tiles
                    xtp = pst.tile([128, 2, 2, 128], F32, tag="xtp")
                    for toff in range(2):
                        t = tp2 * 2 + toff
                        for eh in range(KD):
                            nc.tensor.transpose(
                                xtp[:, toff, eh, :],
                                xn_sb[:, b, t, eh * 128:(eh + 1) * 128].bitcast(F32),
                                ident128[:],
                            )
                    # dest [p, eh, s(2x128)]; src permuted [p, eh, toff, 128]
                    nc.vector.tensor_copy(
                        xt_sb[:, :, b, tp2 * 256:(tp2 + 1) * 256]
                        .rearrange("p k (o s) -> p k o s", o=2),
                        xtp[:].rearrange("p o k s -> p k o s"),
                    )

            # ---- scores[s, h] per (b, s-tile) = xt_tile.T @ q_proj ----
            # out[s, h] = sum_e xt[e, s] * qp[e, h]; exact fp32 (N=8 so cheap)
            scores_ps = ps.tile([128, BL, ST, H], F32, tag="scores")
            for b in range(BL):
                for t in range(ST):
                    for k in range(KD):
                        nc.tensor.matmul(
                            scores_ps[:, b, t, :],
                            xt_sb[:, k, b, t * 128:(t + 1) * 128],
                            qp[:, k, :],
                            start=(k == 0),
                            stop=(k == KD - 1),
                        )
                # exp(scores - SHIFT) -> unnormalized attention weights
                nc.scalar.activation(
                    attn_sb[:, b, :, :],
                    scores_ps[:, b, :, :],
                    mybir.ActivationFunctionType.Exp,
                    bias=negs[:],
                )

            # ---- ctx[h, e] & Z per batch: attnu.T @ [x | 1] (PE) ------
            for b in range(BL):
                ctx_ps = pst.tile([H, 512], F32, tag="ctx")
                for t in range(ST):
                    nc.tensor.matmul(
                        ctx_ps[:, 0:D + 2],
                        attn_sb[:, b, t, :],
                        xn_sb[:, b, t, :],
                        start=(t == 0),
                        stop=(t == ST - 1),
                    )
                # 1/Z from the ones column, then fold into ctx
                nc.vector.reciprocal(recip[:, b, :], ctx_ps[:, D:D + 1])
                nc.vector.tensor_scalar_mul(
                    ctx_sb[:, b, :],
                    ctx_ps[:, 0:D],
                    recip[:, b, :],
                )

            # ---- ctxT[e, (b,h)] via PE transpose ----------------------
            for b in range(BL):
                for eh in range(KD):
                    ctp = pst.tile([128, H], F32, tag="tp")
                    nc.tensor.transpose(
                        ctp[:],
                        ctx_sb[:, b, eh * 128:(eh + 1) * 128],
                        ident[:H, :H],
                    )
                    nc.vector.tensor_copy(ctxT_sb[:, eh, b, :], ctp[:])

            # ---- pooledT[(h d), b] = Wv_h.T @ ctx_h.T (PE, f32r) ------
            pooledT_ps = pst.tile([128, KHD, BL], F32, tag="tp")
            for h in range(H):
                for dh in range(2):
                    for k in range(KD):
                        nc.tensor.matmul(
                            pooledT_ps[:, h * 2 + dh, :],
                            wv_sb[:, k, h * D + dh * 128: h * D + (dh + 1) * 128],
                            ctxT_sb[:, k, :, h],
                            start=(k == 0),
                            stop=(k == KD - 1),
                        )
            nc.vector.tensor_copy(pooledT_sb[:], pooledT_ps[:])

            # ---- bias_total = bv @ Wo + bo (PE) -----------------------
            bvt_ps = pst.tile([128, KHD], F32, tag="tp")
            nc.tensor.transpose(bvt_ps[:], bvn_sb[:], ident[:KHD, :KHD])
            nc.vector.tensor_copy(bvT_sb[:], bvt_ps[:])

            bias_ps = ps.tile([1, D], F32, tag="fin")
            for k in range(KHD):
                nc.tensor.matmul(
                    bias_ps[:],
                    bvT_sb[:, k:k + 1],
                    wo_sb[:, k, :],
                    start=(k == 0),
                    stop=False,
                )
            nc.tensor.matmul(
                bias_ps[:],
                ones_sb[0:1, 0:1],
                bo_sb[:],
                start=False,
                stop=True,
            )
            nc.vector.tensor_copy(bias_sb[:], bias_ps[:])

            # ---- out[b, :] = pooled_flat @ Wo + bias_total (PE, f32r) -
            out_ps = ps.tile([BL, D], F32, tag="scores")
            for k in range(KHD):
                nc.tensor.matmul(
                    out_ps[:],
                    pooledT_sb[:, k, :],
                    wo_sb[:, k, :],
                    start=(k == 0),
                    stop=False,
                )
            nc.tensor.matmul(
                out_ps[:],
                ones_sb[:],
                bias_sb[:],
                start=False,
                stop=True,
            )
            nc.vector.tensor_copy(out_sb[:], out_ps[:])
            nc.sync.dma_start(out_d[:], out_sb[:])

    nc.compile()
    return nc


_NC_CACHE = []


def get_nc():
    if not _NC_CACHE:
        _NC_CACHE.append(build_program())
    return _NC_CACHE[0]


def make_in_maps(x, Wk, bk, Wv, bv, query, Wo, bo):
    x = np.ascontiguousarray(x, dtype=np.float32)
    xn1 = np.concatenate(
        [x, np.ones((x.shape[0], x.shape[1], 2), np.float32)], axis=2
    )
    wk = np.ascontiguousarray(Wk, dtype=np.float32)
    wv = np.ascontiguousarray(Wv, dtype=np.float32)
    wo = np.ascontiguousarray(Wo, dtype=np.float32)
    q = np.ascontiguousarray(query, dtype=np.float32)
    bvv = np.ascontiguousarray(bv, dtype=np.float32)
    bob = np.ascontiguousarray(bo, dtype=np.float32)
    in_maps = []
    for c in range(NCORES):
        sl = slice(c * BL, (c + 1) * BL)
        in_maps.append(
            {
                "xn": xn1[sl],
                "wk": wk,
                "wv": wv,
                "wo": wo,
                "q": q,
                "bv": bvv,
                "bo": bob,
                "on": np.ones((1, BL), np.float32),
            }
        )
    return in_maps


def kernel(x, Wk, bk, Wv, bv, query, Wo, bo):
    nc = get_nc()
    in_maps = make_in_maps(x, Wk, bk, Wv, bv, query, Wo, bo)
    res = run_bass_kernel_spmd(nc, in_maps, core_ids=list(range(NCORES)))
    return np.concatenate([res.results[c]["out"] for c in range(NCORES)], axis=0)



# revision 17
# speedup vs baseline: 2005.3003x; 2005.3003x over previous
"""Trainium2 Bass kernel for CustomMultiHeadAttention (single-query pooled attention).

Reference computation (B=32, S=1024, D=256, H=8):
    keys   = (x @ Wk + bk).reshape(B,S,H,D)
    values = (x @ Wv + bv).reshape(B,S,H,D)
    scores = einsum('bshd,hd->bsh', keys, query)
    attn   = softmax(scores, axis=1)           # over S
    pooled = einsum('bsh,bshd->bhd', attn, values).reshape(B, H*D)
    out    = pooled @ Wo + bo

Algebraic restructure (exact in real arithmetic):
    q_proj[e,h] = sum_d Wk[e, h*D+d] * query[h,d]        # [256, 8]
    scores[b,s,h] = x[b,s,:] @ q_proj[:,h]  (+ const(h) from bk -> cancels in softmax)
    attnu = exp(scores - 64)                             # const shift; softmax invariant
    ctx[b,h,e]  = sum_s attnu[b,s,h] * x[b,s,e];  Z[b,h] = sum_s attnu[b,s,h]
    pooled[b,h,:] = (ctx[b,h,:]/Z[b,h]) @ Wv_h           # sum_s attn = 1
    out = (pooled + bv) @ Wo + bo

This removes both [B*S,256]x[256,2048] projections; the kernel is DMA-bound
(~5.2MB of HBM reads per core vs ~70us of compute spread over 5 engines).
Implementation notes:
  - All wire tensors are fp16 (halves HBM traffic, 10-bit mantissa keeps the
    values path accurate; attn weights use bf16 for exponent range; all
    accumulation is fp32 in PSUM). Host lays tensors out exactly as they sit
    in SBUF so every DMA is fully contiguous.
  - x is shipped once, transposed ([e, s] layout for the scores matmul), in
    4 per-batch DMAs that pipeline against compute. The [s, e] layout the ctx
    matmul needs is rebuilt on-chip with PE transposes; the PSUM->SBUF copies
    alternate between the DVE and Activation engines.
  - q_proj is computed on the PE from a transposed Wk (contract d on
    partitions); Z comes free as two all-ones columns in the rebuilt x.
  - The output projection runs in transposed orientation (Wo stationary,
    pooled moving) and Wo is split into two DMAs, so almost nothing trails
    the last weight byte's arrival. The kernel returns out.T per core
    ([dout, b]); the host transposes during the gather.
  - A dummy Exp at the top preloads the activation table off the critical
    path (Exp/Copy share one table set).
Sharding: data-parallel over batch, 4 batches per core on 8 cores.

build_program(loop_n) wraps the whole body in a hardware For_i loop; kernel()
uses loop_n=1. test.py uses larger loop_n to measure true per-iteration HW
time as a wall-clock slope (the axon dispatch latency cancels).
"""

import sys

sys.path.insert(0, "/opt/trn_rl_repo")

import numpy as np

import concourse.bass as bass
import concourse.mybir as mybir
import concourse.tile as tile
from concourse import bacc
from concourse.bass_utils import run_bass_kernel_spmd
from concourse.masks import make_identity

F32 = mybir.dt.float32
BF16 = mybir.dt.bfloat16
F16 = mybir.dt.float16
NPF16 = mybir.dt.np(F16)

B, S, D, H = 32, 1024, 256, 8
NCORES = 8
BL = B // NCORES      # local batches per core = 4
ST = S // 128         # s-tiles per batch = 8
KD = 2                # 256 = 2 k-tiles of 128 over the D (input dim) axis
KHD = (H * D) // 128  # 16 k-tiles over the H*D axis
SHIFT = 64.0          # constant score shift before exp (softmax-invariant)
MQT, MBV, MBO = 0, KD * H, KD * H + KHD   # misc tensor column offsets
MISC_COLS = KD * H + KHD + KD             # qt | bvt | boT


def build_program(loop_n=1):
    nc = bacc.Bacc("TRN2", target_bir_lowering=False, debug=False)

    xt_d = nc.dram_tensor("xt", [128, KD, BL, S], F16, kind="ExternalInput")
    wkm_d = nc.dram_tensor(
        "wkm", [128, KHD * D + MISC_COLS], F16, kind="ExternalInput"
    )
    wv_d = nc.dram_tensor("wv", [128, KD, H * D], F16, kind="ExternalInput")
    wo_d = nc.dram_tensor("wo", [128, KHD, D], F16, kind="ExternalInput")
    out_d = nc.dram_tensor("out", [128, KD, BL], F32, kind="ExternalOutput")

    with tile.TileContext(nc) as tc:
        with (
            tc.tile_pool(name="big", bufs=1) as big,
            tc.tile_pool(name="sm", bufs=1) as sm,
            tc.tile_pool(name="ps", bufs=1, space=bass.MemorySpace.PSUM) as ps,
            tc.tile_pool(name="pst", bufs=2, space=bass.MemorySpace.PSUM) as pst,
        ):
            # ---- SBUF allocations -------------------------------------
            xt_sb = big.tile([128, KD, BL, S], F16)      # x transposed: p=e%128
            xn_sb = big.tile([128, BL, ST, D + 2], F16)  # x natural + 2 ones cols
            wkm_sb = big.tile([128, KHD * D + MISC_COLS], F16)  # Wk.T | misc
            wv_sb = big.tile([128, KD, H * D], F16)
            wo_sb = big.tile([128, KHD, D], F16)
            wkt_sb = wkm_sb[:, 0:KHD * D].rearrange("p (k d) -> p k d", k=KHD)
            misc_sb = wkm_sb[:, KHD * D:KHD * D + MISC_COLS]
            qp_sb = sm.tile([128, KD, H], F16)           # q_proj [e, h]
            attn_sb = sm.tile([128, BL, ST, H], BF16)    # exp(scores-SHIFT)
            recip = sm.tile([H, BL, 1], F32)             # 1/Z per (h, b)
            ctx_sb = sm.tile([H, BL, D], F16)            # [h, b, e] normalized
            ctxT_sb = sm.tile([128, KD, BL, H], F16)     # [e%128, k, b, h]
            pooledT_sb = sm.tile([128, KHD, BL], F16)    # [(hd)%128, kk, b]
            ident8 = sm.tile([8, 8], F16)
            ident128 = sm.tile([128, 128], F16)
            negs = sm.tile([128, 1], F32)                # -SHIFT bias for exp
            dummy = sm.tile([128, 1], F32)
            outT_sb = sm.tile([128, KD, BL], F32)

            with tc.For_i(0, loop_n):
                # ---- DMA loads (order = transfer order; all contiguous) --
                for b in range(2):
                    nc.sync.dma_start(xt_sb[:, :, b, :], xt_d[:, :, b, :])
                nc.sync.dma_start(wkm_sb[:], wkm_d[:])
                for b in range(2, BL):
                    nc.sync.dma_start(xt_sb[:, :, b, :], xt_d[:, :, b, :])
                nc.sync.dma_start(wv_sb[:], wv_d[:])
                for lo, hi in ((0, 7), (7, 14), (14, 16)):
                    nc.sync.dma_start(
                        wo_sb[:, lo:hi, :], wo_d[:, lo:hi, :]
                    )

                make_identity(nc, ident8[:])
                make_identity(nc, ident128[:])
                nc.vector.memset(negs[:], -SHIFT)
                nc.vector.memset(xn_sb[:, :, :, D:D + 2], 1.0)
                # dummy exp: pull the act-table load off the critical path
                nc.scalar.activation(
                    dummy[:], negs[:], mybir.ActivationFunctionType.Exp
                )

                def emit_transposes(b):
                    # PE transposes of xt -> natural x; 4 share one PSUM bank;
                    # PSUM->SBUF copies spread over DVE/DVE/Pool/ACT so the
                    # ACT queue stays nearly free for the exps
                    for tp2 in range(ST // 2):
                        xtp = pst.tile([128, 2, KD, 128], F16, tag="xtp")
                        for toff in range(2):
                            t = tp2 * 2 + toff
                            for k in range(KD):
                                nc.tensor.transpose(
                                    xtp[:, toff, k, :],
                                    xt_sb[:, k, b, t * 128:(t + 1) * 128],
                                    ident128[:],
                                )
                        dst = (
                            xn_sb[:, b, tp2 * 2:(tp2 + 1) * 2, 0:D]
                            .rearrange("p t (k c) -> p t k c", k=KD)
                        )
                        if tp2 < 3:
                            nc.vector.tensor_copy(dst, xtp[:])
                        else:
                            nc.scalar.activation(
                                dst, xtp[:],
                                mybir.ActivationFunctionType.Copy,
                            )

                def emit_scores(b, scores_ps):
                    for t in range(ST):
                        for k in range(KD):
                            nc.tensor.matmul(
                                scores_ps[:, b, t, :],
                                xt_sb[:, k, b, t * 128:(t + 1) * 128],
                                qp_sb[:, k, :],
                                start=(k == 0),
                                stop=(k == KD - 1),
                            )
                    nc.scalar.activation(
                        attn_sb[:, b, :, :],
                        scores_ps[:, b, :, :],
                        mybir.ActivationFunctionType.Exp,
                        bias=negs[:],
                    )

                # b0/b1 transposes only need x (first two DMAs); qp needs Wk
                emit_transposes(0)
                emit_transposes(1)

                # ---- q_proj[e,h] on PE: contract d over partitions -------
                qp_ps = ps.tile([128, KD, H], F32, tag="out")
                for h in range(H):
                    for et in range(KD):
                        for kq in range(KD):
                            nc.tensor.matmul(
                                qp_ps[:, et, h:h + 1],
                                wkt_sb[:, 2 * h + kq, et * 128:(et + 1) * 128],
                                misc_sb[:, MQT + kq * H + h:MQT + kq * H + h + 1],
                                start=(kq == 0),
                                stop=(kq == KD - 1),
                            )
                nc.vector.tensor_copy(qp_sb[:], qp_ps[:])

                scores_ps = ps.tile([128, BL, ST, H], F32, tag="scores")
                emit_scores(0, scores_ps)
                emit_scores(1, scores_ps)
                emit_transposes(2)
                emit_scores(2, scores_ps)
                emit_transposes(3)
                emit_scores(3, scores_ps)

                # ---- ctx[h, e] & Z per batch: attnu.T @ [x | 1] ----------
                for b in range(BL):
                    ctx_ps = pst.tile([H, 512], F32, tag="ctx")
                    for t in range(ST):
                        nc.tensor.matmul(
                            ctx_ps[:, 0:D + 2],
                            attn_sb[:, b, t, :],
                            xn_sb[:, b, t, :],
                            start=(t == 0),
                            stop=(t == ST - 1),
                        )
                    nc.vector.reciprocal(recip[:, b, :], ctx_ps[:, D:D + 1])
                    nc.vector.tensor_scalar_mul(
                        ctx_sb[:, b, :],
                        ctx_ps[:, 0:D],
                        recip[:, b, :],
                    )
                    # ctxT[e, (b,h)] via PE transpose
                    for et in range(KD):
                        ctp = pst.tile([128, H], F16, tag="tp")
                        nc.tensor.transpose(
                            ctp[:],
                            ctx_sb[:, b, et * 128:(et + 1) * 128],
                            ident8[:],
                        )
                        nc.vector.tensor_copy(ctxT_sb[:, et, b, :], ctp[:])

                # ---- pooledT[(h d), b] = Wv_h.T @ ctx_h.T ----------------
                pooledT_ps = pst.tile([128, KHD, BL], F32, tag="tp")
                for h in range(H):
                    for dh in range(2):
                        for k in range(KD):
                            nc.tensor.matmul(
                                pooledT_ps[:, h * 2 + dh, :],
                                wv_sb[:, k, h * D + dh * 128: h * D + (dh + 1) * 128],
                                ctxT_sb[:, k, :, h],
                                start=(k == 0),
                                stop=(k == KD - 1),
                            )
                # fold bv in before the output projection: (pooled+bv) @ Wo
                nc.vector.tensor_add(
                    pooledT_sb[:],
                    pooledT_ps[:],
                    misc_sb[:, MBV:MBV + KHD]
                    .rearrange("p k -> p k ()").broadcast_to([128, KHD, BL]),
                )

                # ---- outT[dout, b] = Wo.T-tiles (stationary) x pooledT ---
                # two PSUM tiles (separate banks) so both 128-row output
                # halves accumulate kk-outer, chasing the 4 wo DMA chunks
                oT0_ps = ps.tile([128, BL], F32, tag="out")
                oT1_ps = ps.tile([128, BL], F32, tag="scores")
                oT_ps = [oT0_ps, oT1_ps]
                for kk in range(KHD):
                    for et in range(KD):
                        nc.tensor.matmul(
                            oT_ps[et][:],
                            wo_sb[:, kk, et * 128:(et + 1) * 128],
                            pooledT_sb[:, kk, :],
                            start=(kk == 0),
                            stop=(kk == KHD - 1),
                        )
                # + bo (transposed layout), f32 out
                for et in range(KD):
                    nc.vector.tensor_add(
                        outT_sb[:, et, :],
                        oT_ps[et][:],
                        misc_sb[:, MBO + et:MBO + et + 1]
                        .broadcast_to([128, BL]),
                    )
                nc.sync.dma_start(out_d[:], outT_sb[:])

    nc.compile()
    return nc


_NC_CACHE = {}


def get_nc(loop_n=1):
    if loop_n not in _NC_CACHE:
        _NC_CACHE[loop_n] = build_program(loop_n)
    return _NC_CACHE[loop_n]


def make_in_maps(x, Wk, bk, Wv, bv, query, Wo, bo):
    x = np.ascontiguousarray(x, dtype=np.float32)
    # weight-side wire tensors (shared across cores)
    wkt = np.ascontiguousarray(
        np.asarray(Wk, np.float32).T.astype(NPF16)
        .reshape(KHD, 128, D).transpose(1, 0, 2)
    )
    wv = np.ascontiguousarray(
        np.asarray(Wv, np.float32).astype(NPF16)
        .reshape(KD, 128, H * D).transpose(1, 0, 2)
    )
    wo = np.ascontiguousarray(
        np.asarray(Wo, np.float32).astype(NPF16)
        .reshape(KHD, 128, D).transpose(1, 0, 2)
    )
    wkm = np.zeros((128, KHD * D + MISC_COLS), NPF16)
    wkm[:, 0:KHD * D] = wkt.reshape(128, KHD * D)
    moff = KHD * D
    wkm[:, moff + MQT:moff + MQT + KD * H] = (
        np.asarray(query, np.float32).T.astype(NPF16).reshape(KD, 128, H)
        .transpose(1, 0, 2).reshape(128, KD * H)
    )
    wkm[:, moff + MBV:moff + MBV + KHD] = (
        np.asarray(bv, np.float32).astype(NPF16).reshape(KHD, 128).T
    )
    wkm[:, moff + MBO:moff + MBO + KD] = (
        np.asarray(bo, np.float32).astype(NPF16).reshape(KD, 128).T
    )
    xbf = x.astype(NPF16)
    in_maps = []
    for c in range(NCORES):
        xs = xbf[c * BL:(c + 1) * BL]                      # [BL, S, D]
        xt = np.ascontiguousarray(
            xs.transpose(2, 0, 1).reshape(KD, 128, BL, S).transpose(1, 0, 2, 3)
        )
        in_maps.append({"xt": xt, "wkm": wkm, "wv": wv, "wo": wo})
    return in_maps


def kernel(x, Wk, bk, Wv, bv, query, Wo, bo):
    nc = get_nc()
    in_maps = make_in_maps(x, Wk, bk, Wv, bv, query, Wo, bo)
    res = run_bass_kernel_spmd(nc, in_maps, core_ids=list(range(NCORES)))
    # per-core output is outT [128, KD, BL]; untranspose to [BL, D]
    return np.concatenate(
        [
            res.results[c]["out"].transpose(2, 1, 0).reshape(BL, D)
            for c in range(NCORES)
        ],
        axis=0,
    )


# revision 19
# speedup vs baseline: 2258.7716x; 1.1264x over previous
"""Trainium2 Bass kernel for CustomMultiHeadAttention (single-query pooled attention).

Reference computation (B=32, S=1024, D=256, H=8):
    keys   = (x @ Wk + bk).reshape(B,S,H,D)
    values = (x @ Wv + bv).reshape(B,S,H,D)
    scores = einsum('bshd,hd->bsh', keys, query)
    attn   = softmax(scores, axis=1)           # over S
    pooled = einsum('bsh,bshd->bhd', attn, values).reshape(B, H*D)
    out    = pooled @ Wo + bo

Algebraic restructure (exact in real arithmetic):
    q_proj[e,h] = sum_d Wk[e, h*D+d] * query[h,d]        # [256, 8]
    scores[b,s,h] = x[b,s,:] @ q_proj[:,h]  (+ const(h) from bk -> cancels in softmax)
    attnu = exp(scores - 64)                             # const shift; softmax invariant
    ctx[b,h,e]  = sum_s attnu[b,s,h] * x[b,s,e];  Z[b,h] = sum_s attnu[b,s,h]
    pooled[b,h,:] = (ctx[b,h,:]/Z[b,h]) @ Wv_h           # sum_s attn = 1
    out = (pooled + bv) @ Wo + bo

This removes both [B*S,256]x[256,2048] projections. The kernel is DMA-bound:
~7.3MB of HBM reads per core against ~13us of PE work and a few us on the
other engines. Implementation notes:
  - All wire tensors are fp16 (halves HBM traffic; 10-bit mantissa keeps the
    values path accurate; attn weights use bf16 for exponent range; all
    matmul accumulation is fp32 in PSUM). The host lays every tensor out
    exactly as it sits in SBUF, so each DMA is one fully contiguous transfer.
  - x is shipped in BOTH orientations ([e,s] for the scores matmul, [s,e]
    (+ two ones columns) for the ctx matmul), in per-batch DMAs that
    pipeline against compute. Measured on HW this beats rebuilding one
    orientation with PE transposes: the PE was the bottleneck, not DMA.
  - q_proj is computed on the PE from a transposed Wk (contract d on
    partitions); Z comes free from the ones columns in the ctx matmul.
  - The output projection runs in transposed orientation (Wo stationary,
    pooled moving); Wo is split into three DMA chunks with a small last
    chunk, so almost nothing trails the last weight byte's arrival. The
    kernel returns out.T per core ([dout, b]); the host transposes during
    the gather.
  - A dummy Exp at the top preloads the activation table off the critical
    path.
Sharding: data-parallel over batch, 4 batches per core on 8 cores.

build_program(loop_n) wraps the whole body in a hardware For_i loop; kernel()
uses loop_n=1. test.py uses larger loop_n to measure true per-iteration HW
time as a wall-clock slope (the ~70ms axon dispatch latency cancels).
"""

import sys

sys.path.insert(0, "/opt/trn_rl_repo")

import numpy as np

import concourse.bass as bass
import concourse.mybir as mybir
import concourse.tile as tile
from concourse import bacc
from concourse.bass_utils import run_bass_kernel_spmd
from concourse.masks import make_identity

F32 = mybir.dt.float32
BF16 = mybir.dt.bfloat16
F16 = mybir.dt.float16
NPF16 = mybir.dt.np(F16)

B, S, D, H = 32, 1024, 256, 8
NCORES = 8
BL = B // NCORES      # local batches per core = 4
ST = S // 128         # s-tiles per batch = 8
KD = 2                # 256 = 2 k-tiles of 128 over the D (input dim) axis
KHD = (H * D) // 128  # 16 k-tiles over the H*D axis
SHIFT = 64.0          # constant score shift before exp (softmax-invariant)
MQT, MBV, MBO = 0, KD * H, KD * H + KHD   # misc tensor column offsets
MISC_COLS = KD * H + KHD + KD             # qt | bvt | boT


def build_program(loop_n=1):
    nc = bacc.Bacc("TRN2", target_bir_lowering=False, debug=False)

    xt_d = nc.dram_tensor("xt", [128, KD, BL, S], F16, kind="ExternalInput")
    xn_d = nc.dram_tensor("xn", [128, BL, ST, D + 2], F16, kind="ExternalInput")
    wkm_d = nc.dram_tensor(
        "wkm", [128, KHD * D + MISC_COLS], F16, kind="ExternalInput"
    )
    wv_d = nc.dram_tensor("wv", [128, KD, H * D], F16, kind="ExternalInput")
    wo_d = nc.dram_tensor("wo", [128, KHD, D], F16, kind="ExternalInput")
    out_d = nc.dram_tensor("out", [128, KD, BL], F32, kind="ExternalOutput")

    with tile.TileContext(nc) as tc:
        with (
            tc.tile_pool(name="big", bufs=1) as big,
            tc.tile_pool(name="sm", bufs=1) as sm,
            tc.tile_pool(name="ps", bufs=1, space=bass.MemorySpace.PSUM) as ps,
            tc.tile_pool(name="pst", bufs=2, space=bass.MemorySpace.PSUM) as pst,
        ):
            # ---- SBUF allocations -------------------------------------
            xt_sb = big.tile([128, KD, BL, S], F16)      # x transposed: p=e%128
            xn_sb = big.tile([128, BL, ST, D + 2], F16)  # x natural + ones cols
            wkm_sb = big.tile([128, KHD * D + MISC_COLS], F16)  # Wk.T | misc
            wv_sb = big.tile([128, KD, H * D], F16)
            wo_sb = big.tile([128, KHD, D], F16)
            wkt_sb = wkm_sb[:, 0:KHD * D].rearrange("p (k d) -> p k d", k=KHD)
            misc_sb = wkm_sb[:, KHD * D:KHD * D + MISC_COLS]
            qp_sb = sm.tile([128, KD, H], F16)           # q_proj [e, h]
            attn_sb = sm.tile([128, BL, ST, H], BF16)    # exp(scores-SHIFT)
            recip = sm.tile([H, BL, 1], F32)             # 1/Z per (h, b)
            ctx_sb = sm.tile([H, BL, D], F16)            # [h, b, e] normalized
            ctxT_sb = sm.tile([128, KD, BL, H], F16)     # [e%128, k, b, h]
            pooledT_sb = sm.tile([128, KHD, BL], F16)    # [(hd)%128, kk, b]
            ident8 = sm.tile([8, 8], F16)
            negs = sm.tile([128, 1], F32)                # -SHIFT bias for exp
            dummy = sm.tile([128, 1], F32)
            outT_sb = sm.tile([128, KD, BL], F32)

            with tc.For_i(0, loop_n):
                # ---- DMA loads (order = transfer order; all contiguous) --
                nc.sync.dma_start(wkm_sb[:], wkm_d[:])
                for b in range(BL):
                    nc.sync.dma_start(xt_sb[:, :, b, :], xt_d[:, :, b, :])
                nc.sync.dma_start(wv_sb[:], wv_d[:])
                for b in range(BL):
                    nc.sync.dma_start(xn_sb[:, b, :, :], xn_d[:, b, :, :])
                for lo, hi in ((0, 7), (7, 14), (14, 16)):
                    nc.sync.dma_start(
                        wo_sb[:, lo:hi, :], wo_d[:, lo:hi, :]
                    )

                make_identity(nc, ident8[:])
                nc.vector.memset(negs[:], -SHIFT)
                # dummy exp: pull the act-table load off the critical path
                nc.scalar.activation(
                    dummy[:], negs[:], mybir.ActivationFunctionType.Exp
                )

                # ---- q_proj[e,h] on PE: contract d over partitions -------
                qp_ps = ps.tile([128, KD, H], F32, tag="out")
                for h in range(H):
                    for et in range(KD):
                        for kq in range(KD):
                            nc.tensor.matmul(
                                qp_ps[:, et, h:h + 1],
                                wkt_sb[:, 2 * h + kq, et * 128:(et + 1) * 128],
                                misc_sb[:, MQT + kq * H + h:MQT + kq * H + h + 1],
                                start=(kq == 0),
                                stop=(kq == KD - 1),
                            )
                nc.vector.tensor_copy(qp_sb[:], qp_ps[:])

                # ---- scores[s, h] per (b, t) = xt_tile.T @ q_proj --------
                scores_ps = ps.tile([128, BL, ST, H], F32, tag="scores")
                for b in range(BL):
                    for t in range(ST):
                        for k in range(KD):
                            nc.tensor.matmul(
                                scores_ps[:, b, t, :],
                                xt_sb[:, k, b, t * 128:(t + 1) * 128],
                                qp_sb[:, k, :],
                                start=(k == 0),
                                stop=(k == KD - 1),
                            )
                    nc.scalar.activation(
                        attn_sb[:, b, :, :],
                        scores_ps[:, b, :, :],
                        mybir.ActivationFunctionType.Exp,
                        bias=negs[:],
                    )

                # ---- ctx[h, e] & Z per batch: attnu.T @ [x | 1] ----------
                for b in range(BL):
                    ctx_ps = pst.tile([H, 512], F32, tag="ctx")
                    for t in range(ST):
                        nc.tensor.matmul(
                            ctx_ps[:, 0:D + 2],
                            attn_sb[:, b, t, :],
                            xn_sb[:, b, t, :],
                            start=(t == 0),
                            stop=(t == ST - 1),
                        )
                    nc.vector.reciprocal(recip[:, b, :], ctx_ps[:, D:D + 1])
                    nc.vector.tensor_scalar_mul(
                        ctx_sb[:, b, :],
                        ctx_ps[:, 0:D],
                        recip[:, b, :],
                    )
                    # ctxT[e, (b,h)] via PE transpose
                    for et in range(KD):
                        ctp = pst.tile([128, H], F16, tag="tp")
                        nc.tensor.transpose(
                            ctp[:],
                            ctx_sb[:, b, et * 128:(et + 1) * 128],
                            ident8[:],
                        )
                        nc.vector.tensor_copy(ctxT_sb[:, et, b, :], ctp[:])

                # ---- pooledT[(h d), b] = Wv_h.T @ ctx_h.T ----------------
                pooledT_ps = pst.tile([128, KHD, BL], F32, tag="tp")
                for h in range(H):
                    for dh in range(2):
                        for k in range(KD):
                            nc.tensor.matmul(
                                pooledT_ps[:, h * 2 + dh, :],
                                wv_sb[:, k, h * D + dh * 128: h * D + (dh + 1) * 128],
                                ctxT_sb[:, k, :, h],
                                start=(k == 0),
                                stop=(k == KD - 1),
                            )
                # fold bv in before the output projection: (pooled+bv) @ Wo
                nc.vector.tensor_add(
                    pooledT_sb[:],
                    pooledT_ps[:],
                    misc_sb[:, MBV:MBV + KHD]
                    .rearrange("p k -> p k ()").broadcast_to([128, KHD, BL]),
                )

                # ---- outT[dout, b] = Wo.T-tiles (stationary) x pooledT ---
                # two PSUM tiles (separate banks) so both 128-row output
                # halves accumulate kk-outer, chasing the wo DMA chunks
                oT0_ps = ps.tile([128, BL], F32, tag="out")
                oT1_ps = ps.tile([128, BL], F32, tag="scores")
                oT_ps = [oT0_ps, oT1_ps]
                for kk in range(KHD):
                    for et in range(KD):
                        nc.tensor.matmul(
                            oT_ps[et][:],
                            wo_sb[:, kk, et * 128:(et + 1) * 128],
                            pooledT_sb[:, kk, :],
                            start=(kk == 0),
                            stop=(kk == KHD - 1),
                        )
                # + bo (transposed layout), f32 out
                for et in range(KD):
                    nc.vector.tensor_add(
                        outT_sb[:, et, :],
                        oT_ps[et][:],
                        misc_sb[:, MBO + et:MBO + et + 1]
                        .broadcast_to([128, BL]),
                    )
                nc.sync.dma_start(out_d[:], outT_sb[:])

    nc.compile()
    return nc


_NC_CACHE = {}


def get_nc(loop_n=1):
    if loop_n not in _NC_CACHE:
        _NC_CACHE[loop_n] = build_program(loop_n)
    return _NC_CACHE[loop_n]


def make_in_maps(x, Wk, bk, Wv, bv, query, Wo, bo):
    x = np.ascontiguousarray(x, dtype=np.float32)
    # weight-side wire tensors (shared across cores)
    wkt = np.ascontiguousarray(
        np.asarray(Wk, np.float32).T.astype(NPF16)
        .reshape(KHD, 128, D).transpose(1, 0, 2)
    )
    wv = np.ascontiguousarray(
        np.asarray(Wv, np.float32).astype(NPF16)
        .reshape(KD, 128, H * D).transpose(1, 0, 2)
    )
    wo = np.ascontiguousarray(
        np.asarray(Wo, np.float32).astype(NPF16)
        .reshape(KHD, 128, D).transpose(1, 0, 2)
    )
    wkm = np.zeros((128, KHD * D + MISC_COLS), NPF16)
    wkm[:, 0:KHD * D] = wkt.reshape(128, KHD * D)
    moff = KHD * D
    wkm[:, moff + MQT:moff + MQT + KD * H] = (
        np.asarray(query, np.float32).T.astype(NPF16).reshape(KD, 128, H)
        .transpose(1, 0, 2).reshape(128, KD * H)
    )
    wkm[:, moff + MBV:moff + MBV + KHD] = (
        np.asarray(bv, np.float32).astype(NPF16).reshape(KHD, 128).T
    )
    wkm[:, moff + MBO:moff + MBO + KD] = (
        np.asarray(bo, np.float32).astype(NPF16).reshape(KD, 128).T
    )
    xbf = x.astype(NPF16)
    in_maps = []
    for c in range(NCORES):
        xs = xbf[c * BL:(c + 1) * BL]                      # [BL, S, D]
        xt = np.ascontiguousarray(
            xs.transpose(2, 0, 1).reshape(KD, 128, BL, S).transpose(1, 0, 2, 3)
        )
        # natural layout + two all-ones columns, partition = s%128
        xn1 = np.concatenate(
            [xs, np.ones((BL, S, 2), NPF16)], axis=2
        )                                                  # [BL, S, D+2]
        xn = np.ascontiguousarray(
            xn1.reshape(BL, ST, 128, D + 2).transpose(2, 0, 1, 3)
        )
        in_maps.append({"xt": xt, "xn": xn, "wkm": wkm, "wv": wv, "wo": wo})
    return in_maps


def kernel(x, Wk, bk, Wv, bv, query, Wo, bo):
    nc = get_nc()
    in_maps = make_in_maps(x, Wk, bk, Wv, bv, query, Wo, bo)
    res = run_bass_kernel_spmd(nc, in_maps, core_ids=list(range(NCORES)))
    # per-core output is outT [128, KD, BL]; untranspose to [BL, D]
    return np.concatenate(
        [
            res.results[c]["out"].transpose(2, 1, 0).reshape(BL, D)
            for c in range(NCORES)
        ],
        axis=0,
    )
